# revision 1
# baseline (speedup 1.0000x reference)
"""Trainium2 Bass kernel for BidPrefix: per-row cumprod + 3-point gather.

Reference semantics (per row b of inputs [B, 302]):
  rates = inputs[b, :300]; bid = int(inputs[b, 300]); mp = int(inputs[b, 301])
  cpz[k] = prod(rates[:k]) (cpz[0] = 1)
  out[b] = [cpz[bid], cpz[mp+1], cpz[mp]]

Strategy: pure data parallel over 8 NeuronCores. Rows are host-sorted by
max(bid, mp) descending and packed 128-consecutive-sorted-rows per
(core, tile) slot round-robin over cores, so every tap in tile t lies
below a per-tile bound L[t] (hardcoded into the compiled program, cache
keyed on it); streaming past the tap never changes a tap's value, so all
device ops only touch columns [0, L[t]) — about 2/3 of the columns on
average. Per tile the Vector engine runs TWO fused custom DVE ops
(registered at import time):

  TAPCP: accum_out = C1 + sum_k eq(Idx, C0) * cumprod(Src0)[k]

giving cpz[bid] and cpz[mp] in one L-wide pass each (cpz[i] = cp[i-1], so
C0 = idx-1; the idx==0 empty-product case is patched per group on gpsimd,
keeping C1 a literal 0 immediate, which saves an SBUF scalar-operand
read). The third output rides on the otherwise-idle GpSimd and Scalar
engines: the mp-pass's body output is sparse with cp[mp-1] at position
mp-1, so

  cpz[mp+1] = cp[mp] = sum_s body[s] * rates[s+1]

is a gpsimd tensor_tensor multiply of the saved body with the raw tile
shifted by one column, followed by a Scalar-engine activation(Copy) whose
accum_out performs the sum; mp==0 rows (body all zero) are patched per
group with [mp==0] * rates[0]. For the last three (small-L) groups the
Vector engine is DMA-supply limited, so the third tap instead runs as a
third cheap TAPCP on DVE tapping directly at mp (no edge case), which
removes the cross-engine drain at the end of the program. Input rows
arrive via one group-sized DMA (ramped group sizes 2,4,8,16,14,14,
28,...,8,4, split in two chunks, with each group's DMA + scalar prep
emitted two groups ahead of its tile ops) with the bid/mp columns read
strided from that bulk tile; the first two groups source tap scalars
straight from the bulk tile via a -1-folded op variant so the cold-start
critical path avoids the scalar engine's activation-table load. All products reproduce the reference's sequential-f32 cumprod
rounding exactly.
"""

import sys

if "/opt/trn_rl_repo" not in sys.path:
    sys.path.insert(0, "/opt/trn_rl_repo")

import numpy as np

S = 300
COLS = 302
P = 128
NCORES = 8
TILES = 196
BPC = TILES * P  # 25088 rows per core
BTOT = 200000

TRACE = False
LAST_RESULTS = None

_TAP_OPS = None


def _get_tap_ops():
    """Register the fused cumprod+tap custom DVE ops (idempotent).

    TAPCP_ANT:   accum = C1 + sum_k eq(Idx, C0) * cumprod(Src0)[k]
    TAPCPM1_ANT: same with the tap at C0 - 1 (the subtraction is hoisted
                 to latch-init), so s0 can be the raw bid/mp column.
    """
    global _TAP_OPS
    if _TAP_OPS is not None:
        return _TAP_OPS
    import concourse.dve_ops as dve_ops
    from concourse.dve_ops import OPS, DveOp
    from concourse.dve_spec import (
        C0, C1, AluOp, Idx, One, Spec, Src0, eq, lower, scan,
    )
    from concourse.dve_uop import DveOpSpec

    def _make(name, delta, body):
        for op in OPS:
            if op.name == name:
                return op

        def _ref(in0, in1, s0, s1, imm2):
            cp = np.cumprod(in0.astype(np.float32), axis=1, dtype=np.float32)
            n = in0.shape[1]
            k = np.asarray(s0, np.float32).reshape(-1, 1) + np.float32(delta)
            mask = (
                np.arange(n, dtype=np.float32)[None, :] == k
            ).astype(np.float32)
            bodyv = mask * cp
            accum = np.asarray(s1, np.float32).reshape(-1, 1) + bodyv.sum(
                axis=1, keepdims=True
            )
            return bodyv, accum

        spec = Spec(
            body=body, accum=AluOp.ADD, accum_init=C1, reference=_ref,
        )
        shas = {}
        for ver in ("v3", "v4"):
            u = lower(spec, ver=ver)
            shas[ver] = DveOpSpec(
                name=name, opcode=0, uops=u, rd1_en=False
            ).sha(ver)
        op = DveOp(name, spec, subdim=False, uops_sha=shas)
        OPS.append(op)
        dve_ops._SUB_OPCODE_FOR_NAME[name] = (
            dve_ops._CUSTOM_DVE_ROW_BASE + len(OPS) - 1
        )
        dve_ops.CUSTOM_DVE_SPECS[name] = spec
        return op

    tap = _make(
        "TAPCP_ANT", 0.0,
        eq(Idx, C0) * scan(AluOp.MULTIPLY, Src0),
    )
    tapm1 = _make(
        "TAPCPM1_ANT", -1.0,
        eq(Idx, C0 - One) * scan(AluOp.MULTIPLY, Src0),
    )
    _TAP_OPS = (tap, tapm1)
    return _TAP_OPS


def build_nc(tiles=TILES, group=28, L_list=None):
    import concourse.bacc as bacc
    import concourse.mybir as mybir
    from concourse import tile

    f32 = mybir.dt.float32
    A = mybir.AluOpType
    TAP, TAPM1 = _get_tap_ops()

    if L_list is None:
        L_list = [S] * tiles
    bpc = tiles * P
    # ramped group sizes: small first groups so the Vector engine starts
    # before the bulk DMA of a full-size group lands, and a small tail so
    # the last group's cross-engine drain chain is short
    groups = []
    t0 = 0
    for gsz in (2, 4, 8, 16, 14, 14):
        if tiles - t0 > gsz and gsz < group:
            groups.append((t0, gsz))
            t0 += gsz
    tail = [g for g in (8, 4) if g < group]
    ntail = sum(tail)
    while t0 < tiles - ntail:
        gsz = min(group, tiles - ntail - t0)
        groups.append((t0, gsz))
        t0 += gsz
    for gsz in tail:
        if t0 < tiles:
            gsz = min(gsz, tiles - t0)
            groups.append((t0, gsz))
            t0 += gsz

    nc = bacc.Bacc("TRN2", target_bir_lowering=False, debug=False)
    inp = nc.dram_tensor("inp", [bpc, COLS], f32, kind="ExternalInput")
    out = nc.dram_tensor("out", [bpc, 3], f32, kind="ExternalOutput")

    # row = p*tiles + t (partition-major) so group output DMAs coalesce
    vin = inp.ap().rearrange("(p t) c -> p t c", p=P)
    vout = out.ap().rearrange("(p t) k -> p t k", p=P)

    with tile.TileContext(nc) as tc:
        with (
            tc.tile_pool(name="raw", bufs=4) as rawp,
            tc.tile_pool(name="body", bufs=14) as bodyp,
            tc.tile_pool(name="junk", bufs=1) as junkp,
            tc.tile_pool(name="res", bufs=4) as resp,
            tc.tile_pool(name="grp", bufs=4) as grpp,
        ):
            junk = junkp.tile([P, S], mybir.dt.uint8)
            junkA = junkp.tile([P, S], f32, tag="junkA")

            # software-pipelined prep: group g's input DMA + scalar-engine
            # prep (im1 = idx-1, ind0 = [idx==0], read strided out of graw)
            # is emitted two groups ahead of its tile ops, so the Act-queue
            # prep never sits behind the previous group's per-tile sums
            prepped = {}

            def emit_prep(gj):
                t0, gsz = groups[gj]
                cold = gj < 2
                grawT = rawp.tile([P, group, COLS], f32, tag="raw")
                graw = grawT[:, :gsz, :]
                half = (gsz + 1) // 2 if gsz > 2 else gsz
                nc.sync.dma_start(
                    graw[:, :half, :], vin[:, t0 : t0 + half, :]
                )
                if half < gsz:
                    nc.sync.dma_start(
                        graw[:, half:, :], vin[:, t0 + half : t0 + gsz, :]
                    )
                idxf = graw[:, :, S:COLS]
                im1 = None
                if not cold:
                    im1T = grpp.tile([P, group, 2], f32, tag="im1")
                    im1 = im1T[:, :gsz, :]
                    nc.scalar.activation(
                        im1, idxf,
                        mybir.ActivationFunctionType.Copy, bias=-1.0,
                    )
                ind0T = grpp.tile([P, group, 2], f32, tag="ind0")
                ind0 = ind0T[:, :gsz, :]
                nc.scalar.activation(
                    ind0, idxf,
                    mybir.ActivationFunctionType.Relu, bias=1.0, scale=-1.0,
                )
                prepped[gj] = (graw, im1, ind0)

            for gj in range(min(3, len(groups))):
                emit_prep(gj)
            for gi, (t0, gsz) in enumerate(groups):
                if gi + 3 < len(groups):
                    emit_prep(gi + 3)
                dve3 = gi >= len(groups) - 3
                cold = gi < 2
                graw, im1, ind0 = prepped.pop(gi)

                resT = resp.tile([P, group, 3], f32)
                res = resT[:, :gsz, :]
                for ti in range(gsz):
                    raw = graw[:, ti, :]
                    # rows are host-sorted so that this tile's taps all lie
                    # below Lt; streaming past the tap never changes the
                    # accum, so the ops only need columns [0, Lt)
                    Lt = L_list[t0 + ti]
                    rates = raw[:, 0:Lt]

                    # survival = cpz[bid] = cp[bid-1]; bid==0 (+1) patched
                    # per group below. s1 as literal 0 keeps the scalar in
                    # the instruction immediate (no extra SBUF operand read)
                    nc.vector._custom_dve(
                        TAPM1 if cold else TAP,
                        out=junk[:, 0:Lt],
                        in0=rates,
                        s0=raw[:, S : S + 1] if cold else im1[:, ti, 0:1],
                        s1=0.0,
                        accum_out=res[:, ti, 0:1],
                    )
                    if dve3:
                        nc.vector._custom_dve(
                            TAPM1 if cold else TAP,
                            out=junk[:, 0:Lt],
                            in0=rates,
                            s0=raw[:, S + 1 : S + 2] if cold
                            else im1[:, ti, 1:2],
                            s1=0.0,
                            accum_out=res[:, ti, 2:3],
                        )
                        # cpz[mp+1] = cp[mp]: tap directly at mp (needs one
                        # extra column; no mp==0 edge case at all)
                        L3 = min(Lt + 1, S)
                        nc.vector._custom_dve(
                            TAP,
                            out=junk[:, 0:L3],
                            in0=raw[:, 0:L3],
                            s0=raw[:, S + 1 : S + 2],
                            s1=0.0,
                            accum_out=res[:, ti, 1:2],
                        )
                        continue
                    # anlp_last_two = cpz[mp] = cp[mp-1]; mp==0 patched below;
                    # body kept: sparse cp[mp-1] at position mp-1
                    body = bodyp.tile([P, S], f32, tag="body")
                    nc.vector._custom_dve(
                        TAPM1 if cold else TAP,
                        out=body[:, 0:Lt],
                        in0=rates,
                        s0=raw[:, S + 1 : S + 2] if cold
                        else im1[:, ti, 1:2],
                        s1=0.0,
                        accum_out=res[:, ti, 2:3],
                    )
                    # anlp_last_one = cpz[mp+1] = sum_s body[s]*rates[s+1]:
                    # gpsimd multiplies (the column at s+1=Lt is a rate for
                    # Lt<300 and the bid column for Lt=300, where body[299]
                    # is always zero since mp<=299), scalar engine's
                    # activation accumulator does the sum
                    prod = bodyp.tile([P, S], f32, tag="prod")
                    nc.gpsimd.tensor_tensor(
                        prod[:, 0:Lt], body[:, 0:Lt], raw[:, 1 : Lt + 1],
                        A.mult,
                    )
                    nc.scalar.activation(
                        junkA[:, 0:Lt],
                        prod[:, 0:Lt],
                        mybir.ActivationFunctionType.Copy,
                        accum_out=res[:, ti, 1:2],
                    )

                # idx==0 empty-product patches (accums were seeded with 0):
                # res0 += [bid==0]; res2 += [mp==0];
                # res1 += [mp==0]*rates[0] (body was all zero for mp==0)
                nc.gpsimd.tensor_tensor(
                    res[:, :, 0:3:2], res[:, :, 0:3:2], ind0[:, :, 0:2],
                    A.add,
                )
                if not dve3:
                    fixT = grpp.tile([P, group], f32, tag="fix")
                    fix = fixT[:, :gsz]
                    nc.gpsimd.tensor_tensor(
                        fix, ind0[:, :, 1], graw[:, :, 0], A.mult
                    )
                    nc.gpsimd.tensor_tensor(
                        res[:, :, 1], res[:, :, 1], fix, A.add
                    )

                nc.sync.dma_start(vout[:, t0 : t0 + gsz, :], res)

    nc.compile()
    return nc


_NC_CACHE = {}


def _get_nc(L_list):
    key = tuple(L_list)
    if key not in _NC_CACHE:
        _NC_CACHE[key] = build_nc(L_list=list(L_list))
    return _NC_CACHE[key]


def kernel(inputs):
    global LAST_RESULTS
    x = np.ascontiguousarray(np.asarray(inputs), dtype=np.float32)
    assert x.shape == (BTOT, COLS), x.shape

    # Sharding strategy: sort rows by max(bid, mp) descending and pack 128
    # consecutive sorted rows per (core, tile) slot round-robin over cores.
    # Every tap in tile t then lies below L[t], so the device ops stream
    # only L[t] of the 300 columns. Pure host-side permutation; the inverse
    # gather restores the original row order afterwards.
    npad = BPC * NCORES - BTOT
    padrows = np.zeros((npad, COLS), dtype=np.float32)
    padrows[:, :S] = 1.0
    xp = np.concatenate([x, padrows], axis=0)

    key = np.maximum(xp[:, S], xp[:, S + 1]).astype(np.int64)
    order = np.argsort(-key, kind="stable")
    nblocks = NCORES * TILES
    src = order.reshape(nblocks, P).reshape(TILES, NCORES, P)
    src_cpt = np.ascontiguousarray(src.transpose(1, 2, 0))  # [core, p, t]
    flat_src = src_cpt.reshape(NCORES, BPC)
    shards = xp[flat_src]  # [NCORES, BPC, COLS], shard row = p*TILES + t

    block_max = key[order].reshape(nblocks, P)[:, 0]
    L_list = np.maximum(block_max.reshape(TILES, NCORES).max(axis=1), 1)
    L_list = [int(v) for v in L_list]

    in_maps = [{"inp": np.ascontiguousarray(shards[c])} for c in range(NCORES)]

    nc = _get_nc(L_list)
    from concourse.bass_utils import run_bass_kernel_spmd

    r = run_bass_kernel_spmd(
        nc, in_maps, core_ids=list(range(NCORES)), trace=TRACE
    )
    LAST_RESULTS = r
    y = np.concatenate([r.results[c]["out"] for c in range(NCORES)], axis=0)
    out = np.empty((NCORES * BPC, 3), dtype=np.float32)
    out[src_cpt.reshape(-1)] = y.reshape(-1, 3)
    return np.ascontiguousarray(out[:BTOT])



# revision 2
# speedup vs baseline: 1.2866x; 1.2866x over previous
"""Trainium2 Bass kernel for BidPrefix: per-row cumprod + 3-point gather.

Reference semantics (per row b of inputs [B, 302]):
  rates = inputs[b, :300]; bid = int(inputs[b, 300]); mp = int(inputs[b, 301])
  cpz[k] = prod(rates[:k]) (cpz[0] = 1)
  out[b] = [cpz[bid], cpz[mp+1], cpz[mp]]

Strategy: pure data parallel over 8 NeuronCores. Rows are host-sorted by
max(bid, mp) descending and packed 128-per-tile so every tap in tile t
lies below a per-tile bound L[t]. Tiles are batched into groups; the host
packs each (partition, tile) page as [bid, mp, rates[mp], 1.0,
rates[0:W]] (W = group max L) in a flat [128, TOT] DRAM layout, so the
per-group DMA is one contiguous slab and only ~2/3 of the rate columns
ever move.

On device, one custom DVE op (PAGETAP_ANT) processes a whole group per
instruction: a 3-state uop FSM (seed / steady / page-step) runs, per
page, pgidx = 0,1,2,..., cp = running product of the streamed page
(which starts with the packed 1.0, so cp[e] = cpz[e] exactly), and an
accumulator R += (pgidx == tap) * cp that is re-seeded at each page
boundary by the hand-written step uop. R rides the BYPASS chain to the
write port with a stride-0 output AP, so the last element written per
page is cpz[tap]. Two passes per group (tap = bid, tap = mp) give
cpz[bid] and cpz[mp]; cpz[mp+1] = cpz[mp] * rates[mp] is one small
GpSimd multiply per group against the packed rates[mp] column (bit-exact
with the reference's sequential f32 cumprod). The leading-1.0 trick
makes bid==0 / mp==0 fall out naturally (cp[0] = 1), so there are no
edge-case patches. The host does layout only (sort, pad, duplicate
rates[mp] into the page header); every multiply happens on device.
"""

import dataclasses
import sys

if "/opt/trn_rl_repo" not in sys.path:
    sys.path.insert(0, "/opt/trn_rl_repo")

import numpy as np

S = 300
COLS = 302
P = 128
NCORES = 8
TILES = 196
BPC = TILES * P  # 25088 rows per core
BTOT = 200000
HDR = 4  # page header: bid, mp, rates[mp], 1.0

TRACE = False
LAST_RESULTS = None

_PAGETAP = None


def _get_pagetap():
    """Register the batched page-tap custom DVE op (idempotent).

    For in0 = [P, S, N] pages x and in1 = per-page tap index t (stride-0
    broadcast), each page computes R_e = sum_{k<=e} [k == t] * cumprod(x)[k]
    with cumprod and R reset at every page boundary; out (stride-0 per
    page) keeps R_N-1 = cumprod(x)[t].
    """
    global _PAGETAP
    if _PAGETAP is not None:
        return _PAGETAP
    import concourse.dve_ops as dve_ops
    from concourse.dve_ops import OPS, DveOp
    from concourse.dve_spec import (
        AluOp, Bin, Latch, Scan, Spec, Src0, Src1, Zero, One, eq,
        _assemble, _build_placement, _build_state_machine, _collect,
        _hoist_stream_invariant_ops, _validate_body, _Stage, PREV,
    )
    from concourse.dve_uop import (
        DveOpSpec, Trigger, OutSel, OutPath, ENABLE, N_LANES, N_STAGES,
    )

    name = "PAGETAP_ANT"
    for op in OPS:
        if op.name == name:
            _PAGETAP = op
            return op

    def _ref(in0, in1, s0, s1, imm2):
        x = in0.astype(np.float32)
        n = x.shape[-1]
        cp = np.cumprod(x, axis=-1, dtype=np.float32)
        tap = np.asarray(in1, np.float32)[..., :1]
        idxs = np.arange(n, dtype=np.float32)
        run = np.cumsum((idxs == tap) * cp, axis=-1, dtype=np.float32)
        return run

    pgidx = Scan(AluOp.ADD, One, init=Bin(AluOp.SUBTRACT, Zero, One))
    cps = Scan(AluOp.MULTIPLY, Src0, init=One)
    spec = Spec(
        body=eq(pgidx, Src1) * cps,
        accum=AluOp.ADD,
        accum_init=Zero,
        reference=_ref,
    )

    def _uops(ver):
        _validate_body(spec, ver)
        spec2 = _hoist_stream_invariant_ops(spec)
        scans = _collect(spec2.body, Scan)
        latches = _collect(spec2.body, Latch)
        p = _build_placement(spec2, scans, N_STAGES[ver], N_LANES[ver])
        states = _build_state_machine(spec2, scans, latches, p)
        assert len(states) == 2, states
        seed, steady = states
        pg2 = [s for s in scans if s.op == AluOp.ADD][0]
        cp2 = [s for s in scans if s.op == AluOp.MULTIPLY][0]
        steady2 = dataclasses.replace(
            steady,
            trigger=(Trigger.SRC_TENSOR_DONE, Trigger.SUB_DIM_DONE, Trigger.NONE),
            next=(0, 2, 0),
        )
        # page-boundary step uop: processes the first element of the new
        # page with the two scans re-seeded (pgidx <- 0, cp <- x) and the
        # accumulator restarted (R <- 0 + body)
        ov = {
            p.node_stage[pg2]: _Stage(AluOp.BYPASS, Zero),
            p.node_stage[cp2]: _Stage(AluOp.BYPASS, Src0),
            p.accum_stage: _Stage(AluOp.ADD, Zero, PREV),
        }
        step = dataclasses.replace(
            steady,
            overrides=ov,
            trigger=(Trigger.SRC_TENSOR_DONE, Trigger.SUB_DIM_DONE, Trigger.COUNT),
            next=(0, 2, 1),
            repeat=1,
        )
        uops = [_assemble(st) for st in (seed, steady2, step)]
        # the running sum rides the BYPASS chain to block 7's ALU_OUT;
        # write it every element (stride-0 out AP keeps the page-final one)
        for u in uops[1:]:
            u.out[OutPath.WR0_LO] = OutSel.ALU_OUT
            u.out_enable[OutPath.WR0_LO] = ENABLE
        return uops

    raw = {ver: _uops(ver) for ver in ("v3", "v4")}

    @dataclasses.dataclass(frozen=True)
    class _RawDveOp(DveOp):
        raw_uops: dict = dataclasses.field(
            default_factory=dict, compare=False, hash=False
        )

        def compile(self, ver):
            sp = DveOpSpec(
                name=self.name,
                opcode=dve_ops.get_dve_sub_opcode(self.name),
                uops=self.raw_uops[ver],
                rd1_en=True,
            )
            sp.validate(ver)
            return sp

    shas = {
        ver: DveOpSpec(name=name, opcode=0, uops=u, rd1_en=True).sha(ver)
        for ver, u in raw.items()
    }
    op = _RawDveOp(name, spec, subdim=True, uops_sha=shas, raw_uops=raw)
    OPS.append(op)
    dve_ops._SUB_OPCODE_FOR_NAME[name] = (
        dve_ops._CUSTOM_DVE_ROW_BASE + len(OPS) - 1
    )
    dve_ops.CUSTOM_DVE_SPECS[name] = spec
    _PAGETAP = op
    return op


def _plan_groups(L_list):
    """Greedy tile grouping: per group, page width = W+HDR where W = max L
    in the group (tiles arrive sorted desc, so W = L[t0]); fill until the
    per-partition element budget is hit. Small ramp-up budgets let the DVE
    start before a full-size DMA lands; a small tail shortens the drain."""
    n = len(L_list)
    budgets = [768, 1536, 3072] + [6144] * n
    groups = []
    t0 = 0
    gi = 0
    while t0 < n:
        budget = budgets[min(gi, len(budgets) - 1)]
        # reserve a short tail: the last ~8 tiles go in two small groups
        W = max(int(L_list[t0]), 1)
        gsz = max(1, budget // (W + HDR))
        gsz = min(gsz, n - t0)
        rem = n - t0 - gsz
        if 0 < rem < 3:
            gsz = max(1, gsz - (3 - rem))
        groups.append((t0, gsz, max(int(L_list[t0]), 1)))
        t0 += gsz
        gi += 1
    # split the final group into a ramp-down if it is large
    t0, gsz, W = groups[-1]
    if gsz >= 12:
        groups[-1] = (t0, gsz - 8, W)
        groups.append((t0 + gsz - 8, 6, max(int(L_list[t0 + gsz - 8]), 1)))
        groups.append((t0 + gsz - 2, 2, max(int(L_list[t0 + gsz - 2]), 1)))
    return groups


def build_nc(L_list, groups=None):
    import concourse.bacc as bacc
    import concourse.mybir as mybir
    from concourse import tile

    f32 = mybir.dt.float32
    A = mybir.AluOpType
    TAP = _get_pagetap()

    if groups is None:
        groups = _plan_groups(L_list)
    ntiles = len(L_list)
    offs = [0]
    for _, gsz, W in groups:
        offs.append(offs[-1] + gsz * (W + HDR))
    TOT = offs[-1]

    nc = bacc.Bacc("TRN2", target_bir_lowering=False, debug=False)
    inp = nc.dram_tensor("inp", [P, TOT], f32, kind="ExternalInput")
    out = nc.dram_tensor("out", [P, ntiles * 3], f32, kind="ExternalOutput")
    vin = inp.ap()
    vout = out.ap()

    with tile.TileContext(nc) as tc:
        with (
            tc.tile_pool(name="raw", bufs=4) as rawp,
            tc.tile_pool(name="res", bufs=4) as resp,
        ):
            prepped = {}

            def prep(gj):
                _, gsz, W = groups[gj]
                g = rawp.tile([P, gsz * (W + HDR)], f32, tag="raw")
                nc.sync.dma_start(g, vin[:, offs[gj] : offs[gj + 1]])
                prepped[gj] = g

            for gj in range(min(3, len(groups))):
                prep(gj)
            for gi, (t0, gsz, W) in enumerate(groups):
                if gi + 3 < len(groups):
                    prep(gi + 3)
                PW = W + HDR
                N = W + 1
                g3 = prepped.pop(gi).rearrange("p (s w) -> p s w", w=PW)
                in0 = g3[:, :, HDR - 1 : PW]  # leading 1.0 + W rates
                res = resp.tile([P, gsz * 3], f32, tag="res")
                r3 = res.rearrange("p (s k) -> p s k", k=3)
                nc.vector._custom_dve(
                    TAP,
                    out=r3[:, :, 0:1].broadcast_to([P, gsz, N]),
                    in0=in0,
                    in1=g3[:, :, 0:1].broadcast_to([P, gsz, N]),
                )
                nc.vector._custom_dve(
                    TAP,
                    out=r3[:, :, 2:3].broadcast_to([P, gsz, N]),
                    in0=in0,
                    in1=g3[:, :, 1:2].broadcast_to([P, gsz, N]),
                )
                # cpz[mp+1] = cpz[mp] * rates[mp] (packed in the page header)
                nc.gpsimd.tensor_tensor(
                    r3[:, :, 1], r3[:, :, 2], g3[:, :, 2], A.mult
                )
                nc.sync.dma_start(vout[:, t0 * 3 : (t0 + gsz) * 3], res)

    nc.compile()
    return nc


def _prepare(x, ncores, tiles):
    """Sort rows by max(bid, mp) desc, pack into per-core flat page layout.

    Returns (arrs [ncores, P, TOT], L_list, groups, src_cpt)."""
    bpc = tiles * P
    npad = bpc * ncores - x.shape[0]
    assert npad >= 0
    if npad:
        padrows = np.zeros((npad, COLS), dtype=np.float32)
        padrows[:, :S] = 1.0
        xp = np.concatenate([x, padrows], axis=0)
    else:
        xp = x

    key = np.maximum(xp[:, S], xp[:, S + 1]).astype(np.int64)
    order = np.argsort(-key, kind="stable")
    nblocks = ncores * tiles
    src = order.reshape(nblocks, P).reshape(tiles, ncores, P)
    src_cpt = np.ascontiguousarray(src.transpose(1, 2, 0))  # [core, p, t]

    block_max = key[order].reshape(nblocks, P)[:, 0]
    L_list = np.maximum(block_max.reshape(tiles, ncores).max(axis=1), 1)
    L_list = [int(v) for v in L_list]
    groups = _plan_groups(L_list)

    rows = xp[src_cpt]  # [ncores, P, tiles, COLS]
    parts = []
    for t0, gsz, W in groups:
        rg = rows[:, :, t0 : t0 + gsz, :]
        blk = np.empty((ncores, P, gsz, W + HDR), dtype=np.float32)
        blk[..., 0] = rg[..., S]
        blk[..., 1] = rg[..., S + 1]
        mp_i = rg[..., S + 1].astype(np.int64)[..., None]
        blk[..., 2] = np.take_along_axis(rg[..., :S], mp_i, axis=-1)[..., 0]
        blk[..., 3] = 1.0
        blk[..., HDR:] = rg[..., :W]
        parts.append(blk.reshape(ncores, P, gsz * (W + HDR)))
    arrs = np.concatenate(parts, axis=2)
    return np.ascontiguousarray(arrs), L_list, groups, src_cpt


_NC_CACHE = {}


def _get_nc(L_list, groups):
    key = tuple(L_list)
    if key not in _NC_CACHE:
        _NC_CACHE[key] = build_nc(L_list, groups)
    return _NC_CACHE[key]


def kernel(inputs):
    global LAST_RESULTS
    x = np.ascontiguousarray(np.asarray(inputs), dtype=np.float32)
    assert x.shape == (BTOT, COLS), x.shape

    arrs, L_list, groups, src_cpt = _prepare(x, NCORES, TILES)
    in_maps = [{"inp": np.ascontiguousarray(arrs[c])} for c in range(NCORES)]

    nc = _get_nc(L_list, groups)
    from concourse.bass_utils import run_bass_kernel_spmd

    r = run_bass_kernel_spmd(
        nc, in_maps, core_ids=list(range(NCORES)), trace=TRACE
    )
    LAST_RESULTS = r
    ys = np.stack(
        [r.results[c]["out"].reshape(P, TILES, 3) for c in range(NCORES)]
    )  # [core, p, t, 3]
    out = np.empty((NCORES * BPC, 3), dtype=np.float32)
    out[src_cpt.reshape(-1)] = ys.reshape(-1, 3)
    return np.ascontiguousarray(out[:BTOT])


# revision 6
# speedup vs baseline: 1.5471x; 1.2025x over previous
"""Trainium2 Bass kernel for BidPrefix: per-row cumprod + 3-point gather.

Reference semantics (per row b of inputs [B, 302]):
  rates = inputs[b, :300]; bid = int(inputs[b, 300]); mp = int(inputs[b, 301])
  cpz[k] = prod(rates[:k]) (cpz[0] = 1)
  out[b] = [cpz[bid], cpz[mp+1], cpz[mp]]

Strategy: pure data parallel over 8 NeuronCores. Rows are host-sorted by
max(bid, mp) descending and packed 128-per-tile so every tap in tile t
lies below a per-tile bound L[t]. Tiles are batched into groups; the host
packs each (partition, tile) page as [bid, mp, rates[mp], 1.0,
rates[0:W]] (W = group max L) in a flat [128, TOT] DRAM layout, so the
per-group DMA is one contiguous slab and only ~2/3 of the rate columns
ever move.

On device, one custom DVE op (PAGETAP_ANT) processes a whole group per
instruction: a 3-state uop FSM (seed / steady / page-step) runs, per
page, pgidx = 0,1,2,..., cp = running product of the streamed page
(which starts with the packed 1.0, so cp[e] = cpz[e] exactly), and an
accumulator R += (pgidx == tap) * cp that is re-seeded at each page
boundary by the hand-written step uop. R rides the BYPASS chain to the
write port with a stride-0 output AP, so the last element written per
page is cpz[tap]. Two passes per group (tap = bid, tap = mp) give
cpz[bid] and cpz[mp]; cpz[mp+1] = cpz[mp] * rates[mp] is one small
GpSimd multiply per group against the packed rates[mp] column (bit-exact
with the reference's sequential f32 cumprod). The leading-1.0 trick
makes bid==0 / mp==0 fall out naturally (cp[0] = 1), so there are no
edge-case patches. The host does layout only (sort, pad, duplicate
rates[mp] into the page header); every multiply happens on device.
"""

import dataclasses
import sys

if "/opt/trn_rl_repo" not in sys.path:
    sys.path.insert(0, "/opt/trn_rl_repo")

import numpy as np

S = 300
COLS = 302
P = 128
NCORES = 8
TILES = 196
BPC = TILES * P  # 25088 rows per core
BTOT = 200000
HDR = 4  # page header: bid, mp, rates[mp], 1.0

TRACE = False
LAST_RESULTS = None

_PAGETAP = None


def _get_pagetap():
    """Register the batched page-tap custom DVE op (idempotent).

    For in0 = [P, S, N] pages x and in1 = per-page tap index t (stride-0
    broadcast), each page computes R_e = sum_{k<=e} [k == t] * cumprod(x)[k]
    with cumprod and R reset at every page boundary; out (stride-0 per
    page) keeps R_N-1 = cumprod(x)[t].
    """
    global _PAGETAP
    if _PAGETAP is not None:
        return _PAGETAP
    import concourse.dve_ops as dve_ops
    from concourse.dve_ops import OPS, DveOp
    from concourse.dve_spec import (
        AluOp, Bin, Latch, Scan, Spec, Src0, Src1, Zero, One, eq,
        _assemble, _build_placement, _build_state_machine, _collect,
        _hoist_stream_invariant_ops, _validate_body, _Stage, PREV,
    )
    from concourse.dve_uop import (
        DveOpSpec, Trigger, OutSel, OutPath, ENABLE, N_LANES, N_STAGES,
    )

    name = "PAGETAP_ANT"
    for op in OPS:
        if op.name == name:
            _PAGETAP = op
            return op

    def _ref(in0, in1, s0, s1, imm2):
        x = in0.astype(np.float32)
        n = x.shape[-1]
        cp = np.cumprod(x, axis=-1, dtype=np.float32)
        tap = np.asarray(in1, np.float32)[..., :1]
        idxs = np.arange(n, dtype=np.float32)
        run = np.cumsum((idxs == tap) * cp, axis=-1, dtype=np.float32)
        return run

    pgidx = Scan(AluOp.ADD, One, init=Bin(AluOp.SUBTRACT, Zero, One))
    cps = Scan(AluOp.MULTIPLY, Src0, init=One)
    spec = Spec(
        body=eq(pgidx, Src1) * cps,
        accum=AluOp.ADD,
        accum_init=Zero,
        reference=_ref,
    )

    def _uops(ver):
        _validate_body(spec, ver)
        spec2 = _hoist_stream_invariant_ops(spec)
        scans = _collect(spec2.body, Scan)
        latches = _collect(spec2.body, Latch)
        p = _build_placement(spec2, scans, N_STAGES[ver], N_LANES[ver])
        states = _build_state_machine(spec2, scans, latches, p)
        assert len(states) == 2, states
        seed, steady = states
        pg2 = [s for s in scans if s.op == AluOp.ADD][0]
        cp2 = [s for s in scans if s.op == AluOp.MULTIPLY][0]
        steady2 = dataclasses.replace(
            steady,
            trigger=(Trigger.SRC_TENSOR_DONE, Trigger.SUB_DIM_DONE, Trigger.NONE),
            next=(0, 2, 0),
        )
        # page-boundary step uop: processes the first element of the new
        # page with the two scans re-seeded (pgidx <- 0, cp <- x) and the
        # accumulator restarted (R <- 0 + body)
        ov = {
            p.node_stage[pg2]: _Stage(AluOp.BYPASS, Zero),
            p.node_stage[cp2]: _Stage(AluOp.BYPASS, Src0),
            p.accum_stage: _Stage(AluOp.ADD, Zero, PREV),
        }
        step = dataclasses.replace(
            steady,
            overrides=ov,
            trigger=(Trigger.SRC_TENSOR_DONE, Trigger.SUB_DIM_DONE, Trigger.COUNT),
            next=(0, 2, 1),
            repeat=1,
        )
        uops = [_assemble(st) for st in (seed, steady2, step)]
        # the running sum rides the BYPASS chain to block 7's ALU_OUT;
        # write it every element (stride-0 out AP keeps the page-final one)
        for u in uops[1:]:
            u.out[OutPath.WR0_LO] = OutSel.ALU_OUT
            u.out_enable[OutPath.WR0_LO] = ENABLE
        return uops

    raw = {ver: _uops(ver) for ver in ("v3", "v4")}

    @dataclasses.dataclass(frozen=True)
    class _RawDveOp(DveOp):
        raw_uops: dict = dataclasses.field(
            default_factory=dict, compare=False, hash=False
        )

        def compile(self, ver):
            sp = DveOpSpec(
                name=self.name,
                opcode=dve_ops.get_dve_sub_opcode(self.name),
                uops=self.raw_uops[ver],
                rd1_en=True,
            )
            sp.validate(ver)
            return sp

    shas = {
        ver: DveOpSpec(name=name, opcode=0, uops=u, rd1_en=True).sha(ver)
        for ver, u in raw.items()
    }
    op = _RawDveOp(name, spec, subdim=True, uops_sha=shas, raw_uops=raw)
    OPS.append(op)
    dve_ops._SUB_OPCODE_FOR_NAME[name] = (
        dve_ops._CUSTOM_DVE_ROW_BASE + len(OPS) - 1
    )
    dve_ops.CUSTOM_DVE_SPECS[name] = spec
    _PAGETAP = op
    return op


def _plan_groups(L_list):
    """Greedy tile grouping: per group, page width = W+1 where W = max L
    in the group (tiles arrive sorted desc, so W = L[t0]); fill until the
    per-partition element budget is hit. Small ramp-up budgets let the DVE
    start before a full-size DMA lands; a small tail shortens the drain."""
    n = len(L_list)
    budgets = [384, 768, 1536, 3072] + [6144] * n
    groups = []
    t0 = 0
    gi = 0
    while t0 < n:
        budget = budgets[min(gi, len(budgets) - 1)]
        W = max(int(L_list[t0]), 1)
        gsz = max(1, budget // (W + HDR))
        gsz = min(gsz, n - t0)
        rem = n - t0 - gsz
        if 0 < rem < 3:
            gsz = max(1, gsz - (3 - rem))
        groups.append((t0, gsz, max(int(L_list[t0]), 1)))
        t0 += gsz
        gi += 1
    # split the final group into a ramp-down if it is large
    t0, gsz, W = groups[-1]
    if gsz >= 12:
        groups[-1] = (t0, gsz - 8, W)
        groups.append((t0 + gsz - 8, 6, max(int(L_list[t0 + gsz - 8]), 1)))
        groups.append((t0 + gsz - 2, 2, max(int(L_list[t0 + gsz - 2]), 1)))
    return groups


def _group_cols(gsz, W):
    """Per-partition f32 slots for one group: 3 header blocks (bid, mp,
    rates[mp]; each [gsz]) + contiguous rate pages [gsz, W+1] (leading 1.0
    + W rates)."""
    return 3 * gsz + gsz * (W + 1)


def build_nc(L_list, groups=None):
    import concourse.bacc as bacc
    import concourse.mybir as mybir
    from concourse import tile

    f32 = mybir.dt.float32
    A = mybir.AluOpType
    TAP = _get_pagetap()

    if groups is None:
        groups = _plan_groups(L_list)
    ntiles = len(L_list)
    offs = [0]
    for _, gsz, W in groups:
        offs.append(offs[-1] + _group_cols(gsz, W))
    TOT = offs[-1]

    nc = bacc.Bacc("TRN2", target_bir_lowering=False, debug=False)
    inp = nc.dram_tensor("inp", [P, TOT], f32, kind="ExternalInput")
    out = nc.dram_tensor("out", [P, ntiles * 3], f32, kind="ExternalOutput")
    vin = inp.ap()
    vout = out.ap()

    with tile.TileContext(nc) as tc:
        with (
            tc.tile_pool(name="raw", bufs=5) as rawp,
            tc.tile_pool(name="res", bufs=5) as resp,
        ):
            prepped = {}

            def prep(gj):
                _, gsz, W = groups[gj]
                g = rawp.tile([P, _group_cols(gsz, W)], f32, tag="raw")
                nc.sync.dma_start(g, vin[:, offs[gj] : offs[gj + 1]])
                prepped[gj] = g

            for gj in range(min(4, len(groups))):
                prep(gj)
            for gi, (t0, gsz, W) in enumerate(groups):
                if gi + 4 < len(groups):
                    prep(gi + 4)
                N = W + 1
                g = prepped.pop(gi)
                rates = g[:, 3 * gsz :].rearrange("p (s w) -> p s w", w=N)
                # res layout [P, 3, gsz]: k-major so every operand below is
                # a contiguous [P, gsz] block (strided APs are slow on the
                # Q7 gpsimd engine)
                res = resp.tile([P, 3 * gsz], f32, tag="res")

                def col(ap, j0, n=gsz):
                    return ap[:, j0 * gsz : j0 * gsz + n]

                nc.vector._custom_dve(
                    TAP,
                    out=col(res, 0).unsqueeze(2).broadcast_to([P, gsz, N]),
                    in0=rates,
                    in1=col(g, 0).unsqueeze(2).broadcast_to([P, gsz, N]),
                )
                nc.vector._custom_dve(
                    TAP,
                    out=col(res, 2).unsqueeze(2).broadcast_to([P, gsz, N]),
                    in0=rates,
                    in1=col(g, 1).unsqueeze(2).broadcast_to([P, gsz, N]),
                )
                # cpz[mp+1] = cpz[mp] * rates[mp] (packed header block)
                nc.gpsimd.tensor_tensor(
                    col(res, 1), col(res, 2), col(g, 2), A.mult
                )
                nc.sync.dma_start(vout[:, t0 * 3 : (t0 + gsz) * 3], res)

    nc.compile()
    return nc


def _prepare(x, ncores, tiles):
    """Sort rows by max(bid, mp) desc, pack into per-core flat page layout.

    Returns (arrs [ncores, P, TOT], L_list, groups, src_cpt)."""
    bpc = tiles * P
    npad = bpc * ncores - x.shape[0]
    assert npad >= 0
    if npad:
        padrows = np.zeros((npad, COLS), dtype=np.float32)
        padrows[:, :S] = 1.0
        xp = np.concatenate([x, padrows], axis=0)
    else:
        xp = x

    key = np.maximum(xp[:, S], xp[:, S + 1]).astype(np.int64)
    order = np.argsort(-key, kind="stable")
    nblocks = ncores * tiles
    src = order.reshape(nblocks, P).reshape(tiles, ncores, P)
    src_cpt = np.ascontiguousarray(src.transpose(1, 2, 0))  # [core, p, t]

    block_max = key[order].reshape(nblocks, P)[:, 0]
    L_list = np.maximum(block_max.reshape(tiles, ncores).max(axis=1), 1)
    L_list = [int(v) for v in L_list]
    groups = _plan_groups(L_list)

    rows = xp[src_cpt]  # [ncores, P, tiles, COLS]
    parts = []
    for t0, gsz, W in groups:
        rg = rows[:, :, t0 : t0 + gsz, :]
        hdr = np.empty((ncores, P, 3, gsz), dtype=np.float32)
        hdr[:, :, 0] = rg[..., S]
        hdr[:, :, 1] = rg[..., S + 1]
        mp_i = rg[..., S + 1].astype(np.int64)[..., None]
        hdr[:, :, 2] = np.take_along_axis(rg[..., :S], mp_i, axis=-1)[..., 0]
        pg = np.empty((ncores, P, gsz, W + 1), dtype=np.float32)
        pg[..., 0] = 1.0
        pg[..., 1:] = rg[..., :W]
        parts.append(hdr.reshape(ncores, P, 3 * gsz))
        parts.append(pg.reshape(ncores, P, gsz * (W + 1)))
    arrs = np.concatenate(parts, axis=2)
    return np.ascontiguousarray(arrs), L_list, groups, src_cpt


_NC_CACHE = {}


def _get_nc(L_list, groups):
    key = tuple(L_list)
    if key not in _NC_CACHE:
        _NC_CACHE[key] = build_nc(L_list, groups)
    return _NC_CACHE[key]


def kernel(inputs):
    global LAST_RESULTS
    x = np.ascontiguousarray(np.asarray(inputs), dtype=np.float32)
    assert x.shape == (BTOT, COLS), x.shape

    arrs, L_list, groups, src_cpt = _prepare(x, NCORES, TILES)
    in_maps = [{"inp": np.ascontiguousarray(arrs[c])} for c in range(NCORES)]

    nc = _get_nc(L_list, groups)
    from concourse.bass_utils import run_bass_kernel_spmd

    r = run_bass_kernel_spmd(
        nc, in_maps, core_ids=list(range(NCORES)), trace=TRACE
    )
    LAST_RESULTS = r
    ys = np.empty((NCORES, P, TILES, 3), dtype=np.float32)
    for c in range(NCORES):
        yc = np.asarray(r.results[c]["out"])  # [P, 3*TILES], k-major blocks
        for t0, gsz, W in groups:
            blk = yc[:, 3 * t0 : 3 * (t0 + gsz)].reshape(P, 3, gsz)
            ys[c, :, t0 : t0 + gsz, :] = blk.transpose(0, 2, 1)
    out = np.empty((NCORES * BPC, 3), dtype=np.float32)
    out[src_cpt.reshape(-1)] = ys.reshape(-1, 3)
    return np.ascontiguousarray(out[:BTOT])
